# revision 8
# baseline (speedup 1.0000x reference)
"""ColBERT MaxSim contrastive loss on 8 Trainium2 NeuronCores.

Sharding: doc-parallel (each core scores ALL 64*32 query tokens against its
8-doc shard = 8192 doc tokens). Per core the work is 128 (m-tile, doc) units
(16 m-tiles of 128 query rows x 8 docs), processed in 64 groups of 2 units:

  - PE: per unit, 2 bf16 matmuls qT[128,128].T @ dT[128,512] -> the unit's
    A-half (tokens 0-511) into the big A PSUM tensor [128, 4, 512] and its
    B-half (tokens 512-1023) into the big B PSUM tensor [128, 4, 512].
    Groups alternate slot pairs {0,1}/{2,3} of each tensor, so the PE fills
    one pair while ScalarE/VectorE drain the other (bank-disjoint).
  - ScalarE: ONE 1024-elem copy per group: B slots -> SBUF.
  - VectorE: ONE custom paged op per group: TT_MAXMAX_PG_ANT streams
    (A_psum[i], B_sbuf[i]) pairs at 1 pair/cycle, keeps a running
    max(pair...) scan that RESETS at each 512-element page (SUB_DIM_DONE
    step state) and writes only page-last values (out_last_subdim_enable)
    -> the 2 units' per-(row, doc) maxes, no accumulator-readout companion.

A dummy matmul per group re-targets the next A slot (overwritten by the real
start=True matmul) purely to keep the PE HAM clock-gate at 8/8.

Host gathers the 8 per-core dmax[128 rows, 128 units] tiles, does the tiny
n-sum over 32 query tokens, length-normalize, and the cross-entropy.
"""

import numpy as np

B, NTOK, DIM = 64, 32, 128
C, S = 64, 1024
NCORES = 8
CSHARD = C // NCORES              # 8 docs per core
ROWS = B * NTOK                   # 2048 score rows
MTILES = ROWS // 128              # 16
DCOLS = CSHARD * S                # 8192 doc-token columns per core
UNITS = MTILES * CSHARD           # 128 (m-tile, doc) units
GROUP = 2                         # units per DVE op / slot pair
NGROUPS = UNITS // GROUP          # 64
TEMPERATURE = 0.02

_CACHE = {}


def _register_ttmax_paged():
    """Custom DVE op: out[p, s] = max_n max(in0[p, s, n], in1[p, s*N + n]).

    Built from Spec(body=Scan(MAX, maxx(Src0, Src1), init=MaxNeg)); the
    lowered [seed, steady] FSM is hand-extended with a SUB_DIM_DONE step
    state that resets the scan flop to the boundary element's pair-max, and
    out_last_subdim_enable so only page-last scan values are written (one
    output per page). Validated bit-exact on hardware for PSUM and SBUF in0.
    """
    import copy

    from concourse import dve_ops as DO
    from concourse.dve_spec import AluOp, MaxNeg, Scan, Spec, Src0, Src1, lower, maxx
    from concourse.dve_uop import AluInp, DveOpSpec, ENABLE, Trigger

    NAME = "TT_MAXMAX_PG_ANT"
    for o in DO.OPS:
        if o.name == NAME:
            return o

    def _ref(in0, in1, c0, c1, c2):
        P = in0.shape[0]
        N = in0.shape[-1]
        Spg = int(np.prod(in0.shape[1:-1]))
        a = np.asarray(in0, np.float32).reshape(P, Spg, N)
        b = np.asarray(in1, np.float32).reshape(P, Spg, N)
        return np.maximum(a, b).max(axis=-1)

    spec = Spec(body=Scan(AluOp.MAX, maxx(Src0, Src1), init=MaxNeg), reference=_ref)
    uops = lower(spec, ver="v3")
    assert len(uops) == 2
    seed, steady = uops
    steady.trigger = (Trigger.SRC_TENSOR_DONE, Trigger.SUB_DIM_DONE, Trigger.NONE)
    steady.next_uop = (0, 2, 0)
    steady.out_last_subdim_enable = ENABLE
    step = copy.deepcopy(steady)
    step.trigger = (Trigger.SRC_TENSOR_DONE, Trigger.SUB_DIM_DONE, Trigger.COUNT)
    step.next_uop = (0, 2, 1)
    step.repeat_count = 1
    dp = step.datapath_config[1]
    dp.op = AluOp.BYPASS
    dp.alu_src0 = AluInp.PREV_ALU_OUT
    dp.alu_src1 = AluInp.PREV_ALU_OUT

    op = DO.DveOp(NAME, spec, subdim=True, uops_sha={})
    DO.OPS.append(op)
    DO.CUSTOM_DVE_SPECS[op.name] = op.spec
    DO._SUB_OPCODE_FOR_NAME[op.name] = DO._CUSTOM_DVE_ROW_BASE + len(DO.OPS) - 1
    ds = DveOpSpec(
        name=NAME,
        opcode=DO.get_dve_sub_opcode(NAME),
        uops=[seed, steady, step],
        rd1_en=True,
    )
    ds.validate("v3")
    op.uops_sha["v3"] = ds.sha("v3")
    DO._COMPILE_CACHE[(NAME, "v3")] = ds
    return op


def _build_nc():
    import concourse.bacc as bacc
    import concourse.tile as tile
    from concourse import mybir

    f32 = mybir.dt.float32
    bf16 = mybir.dt.bfloat16
    op = _register_ttmax_paged()

    nc = bacc.Bacc("TRN2", target_bir_lowering=False, debug=False)
    qT_d = nc.dram_tensor("qT", [DIM, ROWS], bf16, kind="ExternalInput").ap()
    dT_d = nc.dram_tensor("dT", [DIM, DCOLS], bf16, kind="ExternalInput").ap()
    dmax_d = nc.dram_tensor("dmax", [128, UNITS], f32, kind="ExternalOutput").ap()

    def unit_mm(dst, u, half):
        m, d = u // CSHARD, u % CSHARD
        col = d * 1024 + half * 512
        nc.tensor.matmul(
            dst,
            qT_sb[:, m * 128:(m + 1) * 128],
            dT_sb[:, col:col + 512],
            start=True,
            stop=True,
        )

    with tile.TileContext(nc) as tc:
        with (
            tc.tile_pool(name="const", bufs=1) as cpool,
            tc.tile_pool(name="b1", bufs=6) as b1_pool,
            tc.tile_pool(name="pa", bufs=1, space="PSUM") as pa_pool,
            tc.tile_pool(name="pb", bufs=1, space="PSUM") as pb_pool,
        ):
            qT_sb = cpool.tile([DIM, ROWS], bf16)
            dT_sb = cpool.tile([DIM, DCOLS], bf16)
            wsb = cpool.tile([128, 512], bf16)
            # 4 rotating output tiles so consecutive DVE ops have no WAW
            # hazard on a shared tile (a shared tile serializes each op on
            # its predecessor's completion = pipe-drain, ~0.9us/op)
            dmax_sb = [
                cpool.tile([128, UNITS // 4], f32, name=f"dmax{r}") for r in range(4)
            ]
            nc.gpsimd.memset(wsb[:], 0.0)

            A = pa_pool.tile([128, 4, 512], f32)   # A-half slots, banks 0-3
            Bp = pb_pool.tile([128, 4, 512], f32)  # B-half slots, banks 4-7

            # HAM warm-up on zeros first — only needs wsb, runs during the
            # DMA fill (slot 0 is overwritten by a real start=True matmul).
            for _ in range(8):
                nc.tensor.matmul(A[:, 0, :], wsb[:, 0:128], wsb[:], start=True, stop=True)

            # DMA staging: what the first groups need first, then big chunks.
            nc.sync.dma_start(qT_sb[:, 0:128], qT_d[:, 0:128])
            nc.sync.dma_start(dT_sb[:, 0:1024], dT_d[:, 0:1024])
            nc.sync.dma_start(dT_sb[:, 1024:2048], dT_d[:, 1024:2048])
            nc.sync.dma_start(dT_sb[:, 2048:3072], dT_d[:, 2048:3072])
            nc.gpsimd.dma_start(qT_sb[:, 128:2048], qT_d[:, 128:2048])
            nc.sync.dma_start(dT_sb[:, 3072:4096], dT_d[:, 3072:4096])
            nc.sync.dma_start(dT_sb[:, 4096:6144], dT_d[:, 4096:6144])
            nc.sync.dma_start(dT_sb[:, 6144:8192], dT_d[:, 6144:8192])

            # software pipeline: B-half matmuls + copy run one group ahead
            def emit_b(g):
                s = (g % 2) * 2
                for j in range(GROUP):
                    unit_mm(Bp[:, s + j, :], g * GROUP + j, 1)
                t = b1_pool.tile([128, GROUP * 512], f32, tag="b1")
                nc.scalar.copy(t[:], Bp[:, s:s + GROUP, :])
                return t

            in1_next = emit_b(0)
            for g in range(NGROUPS):
                s = (g % 2) * 2
                in1 = in1_next
                for j in range(GROUP):
                    unit_mm(A[:, s + j, :], g * GROUP + j, 0)
                if g + 1 < NGROUPS:
                    in1_next = emit_b(g + 1)
                    # warm-keeper: dummy into the next group's first A slot;
                    # the real matmul overwrites it (start=True)
                    nc.tensor.matmul(
                        A[:, 2 - s, :], wsb[:, 0:128], wsb[:], start=True, stop=True
                    )
                r, c = g % 4, (g // 4) * GROUP
                nc.vector._custom_dve(
                    op,
                    out=dmax_sb[r][:, c:c + GROUP],
                    in0=A[:, s:s + GROUP, :],
                    in1=in1[:],
                )

            for r in range(4):
                nc.sync.dma_start(
                    dmax_d[:, r * (UNITS // 4):(r + 1) * (UNITS // 4)], dmax_sb[r][:]
                )

    nc.compile()
    return nc


def _host_inputs(q, d):
    import ml_dtypes

    bf = ml_dtypes.bfloat16
    qT = np.ascontiguousarray(q.transpose(2, 0, 1).reshape(DIM, ROWS)).astype(bf)
    in_maps = []
    for k in range(NCORES):
        dTk = np.ascontiguousarray(
            d[k * CSHARD:(k + 1) * CSHARD].transpose(2, 0, 1).reshape(DIM, DCOLS)
        ).astype(bf)
        in_maps.append({"qT": qT, "dT": dTk})
    return in_maps


def _finish_host(dmaxes, q, offset):
    # dmax[k]: [128 rows, 128 cols], op g (units 2g, 2g+1) wrote cols
    # (g%4)*32 + (g//4)*2 + {0,1}; de-interleave to unit order u = m*8 + d
    g = np.arange(NGROUPS)
    src = (g % 4) * (UNITS // 4) + (g // 4) * GROUP
    perm = np.empty(UNITS, np.int64)
    perm[2 * g] = src
    perm[2 * g + 1] = src + 1
    per_core = []
    for k in range(NCORES):
        m_r_d = dmaxes[k][:, perm].astype(np.float64).reshape(128, MTILES, CSHARD)
        rows_d = m_r_d.transpose(1, 0, 2).reshape(ROWS, CSHARD)
        per_core.append(rows_d.reshape(B, NTOK, CSHARD).sum(axis=1))
    S_mat = np.concatenate(per_core, axis=1)  # [64, 64]
    lengths = (q[:, :, 0] != 0).sum(axis=1).astype(np.float64)
    S_mat = S_mat / lengths[:, None]
    logits = S_mat / TEMPERATURE
    m = logits.max(axis=1, keepdims=True)
    logp = logits - m - np.log(np.exp(logits - m).sum(axis=1, keepdims=True))
    labels = np.arange(B) + offset
    return np.float32(-np.mean(logp[np.arange(B), labels]))


def kernel(**inputs):
    from concourse import bass_utils

    q = np.ascontiguousarray(np.asarray(inputs["query_embeddings"], dtype=np.float32))
    d = np.ascontiguousarray(np.asarray(inputs["doc_embeddings"], dtype=np.float32))
    offset = int(np.asarray(inputs["offset"]))
    assert q.shape == (B, NTOK, DIM) and d.shape == (C, S, DIM)

    if "nc" not in _CACHE:
        _CACHE["nc"] = _build_nc()
    nc = _CACHE["nc"]

    in_maps = _host_inputs(q, d)
    res = bass_utils.run_bass_kernel_spmd(nc, in_maps, core_ids=list(range(NCORES)))
    dmaxes = [res.results[k]["dmax"] for k in range(NCORES)]
    return _finish_host(dmaxes, q, offset)


# revision 10
# speedup vs baseline: 1.1085x; 1.1085x over previous
"""ColBERT MaxSim contrastive loss on 8 Trainium2 NeuronCores.

Sharding: doc-parallel (each core scores ALL 64*32 query tokens against its
8-doc shard = 8192 doc tokens). Per core the work is 128 (m-tile, doc) units
(16 m-tiles of 128 query rows x 8 docs), processed in 64 groups of 2 units:

  - PE: per unit, 2 bf16 matmuls qT[128,128].T @ dT[128,512] -> the unit's
    A-half (tokens 0-511) into the big A PSUM tensor [128, 4, 512] and its
    B-half (tokens 512-1023) into the big B PSUM tensor [128, 4, 512].
    Groups alternate slot pairs {0,1}/{2,3} of each tensor, so the PE fills
    one pair while ScalarE/VectorE drain the other (bank-disjoint).
  - ScalarE: ONE 1024-elem copy per group: B slots -> SBUF.
  - VectorE: ONE custom paged op per group: TT_MAXMAX_PG_ANT streams
    (A_psum[i], B_sbuf[i]) pairs at 1 pair/cycle, keeps a running
    max(pair...) scan that RESETS at each 512-element page (SUB_DIM_DONE
    step state) and writes only page-last values (out_last_subdim_enable)
    -> the 2 units' per-(row, doc) maxes, no accumulator-readout companion.

A dummy matmul per group re-targets the next A slot (overwritten by the real
start=True matmul) purely to keep the PE HAM clock-gate at 8/8.

Host gathers the 8 per-core dmax[128 rows, 128 units] tiles, does the tiny
n-sum over 32 query tokens, length-normalize, and the cross-entropy.
"""

import numpy as np

B, NTOK, DIM = 64, 32, 128
C, S = 64, 1024
NCORES = 8
CSHARD = C // NCORES              # 8 docs per core
ROWS = B * NTOK                   # 2048 score rows
MTILES = ROWS // 128              # 16
DCOLS = CSHARD * S                # 8192 doc-token columns per core
UNITS = MTILES * CSHARD           # 128 (m-tile, doc) units
GROUP = 2                         # units per DVE op / slot pair
NGROUPS = UNITS // GROUP          # 64
TEMPERATURE = 0.02

_CACHE = {}


def _register_ttmax_paged():
    """Custom DVE op: out[p, s] = max_n max(in0[p, s, n], in1[p, s*N + n]).

    Built from Spec(body=Scan(MAX, maxx(Src0, Src1), init=MaxNeg)); the
    lowered [seed, steady] FSM is hand-extended with a SUB_DIM_DONE step
    state that resets the scan flop to the boundary element's pair-max, and
    out_last_subdim_enable so only page-last scan values are written (one
    output per page). Validated bit-exact on hardware for PSUM and SBUF in0.
    """
    import copy

    from concourse import dve_ops as DO
    from concourse.dve_spec import AluOp, MaxNeg, Scan, Spec, Src0, Src1, lower, maxx
    from concourse.dve_uop import AluInp, DveOpSpec, ENABLE, Trigger

    NAME = "TT_MAXMAX_PG_ANT"
    for o in DO.OPS:
        if o.name == NAME:
            return o

    def _ref(in0, in1, c0, c1, c2):
        P = in0.shape[0]
        N = in0.shape[-1]
        Spg = int(np.prod(in0.shape[1:-1]))
        a = np.asarray(in0, np.float32).reshape(P, Spg, N)
        b = np.asarray(in1, np.float32).reshape(P, Spg, N)
        return np.maximum(a, b).max(axis=-1)

    spec = Spec(body=Scan(AluOp.MAX, maxx(Src0, Src1), init=MaxNeg), reference=_ref)
    uops = lower(spec, ver="v3")
    assert len(uops) == 2
    seed, steady = uops
    steady.trigger = (Trigger.SRC_TENSOR_DONE, Trigger.SUB_DIM_DONE, Trigger.NONE)
    steady.next_uop = (0, 2, 0)
    steady.out_last_subdim_enable = ENABLE
    step = copy.deepcopy(steady)
    step.trigger = (Trigger.SRC_TENSOR_DONE, Trigger.SUB_DIM_DONE, Trigger.COUNT)
    step.next_uop = (0, 2, 1)
    step.repeat_count = 1
    dp = step.datapath_config[1]
    dp.op = AluOp.BYPASS
    dp.alu_src0 = AluInp.PREV_ALU_OUT
    dp.alu_src1 = AluInp.PREV_ALU_OUT

    op = DO.DveOp(NAME, spec, subdim=True, uops_sha={})
    DO.OPS.append(op)
    DO.CUSTOM_DVE_SPECS[op.name] = op.spec
    DO._SUB_OPCODE_FOR_NAME[op.name] = DO._CUSTOM_DVE_ROW_BASE + len(DO.OPS) - 1
    ds = DveOpSpec(
        name=NAME,
        opcode=DO.get_dve_sub_opcode(NAME),
        uops=[seed, steady, step],
        rd1_en=True,
    )
    ds.validate("v3")
    op.uops_sha["v3"] = ds.sha("v3")
    DO._COMPILE_CACHE[(NAME, "v3")] = ds
    return op


def _build_nc():
    import concourse.bacc as bacc
    import concourse.tile as tile
    from concourse import mybir

    f32 = mybir.dt.float32
    bf16 = mybir.dt.bfloat16
    op = _register_ttmax_paged()

    nc = bacc.Bacc("TRN2", target_bir_lowering=False, debug=False)
    qT_d = nc.dram_tensor("qT", [DIM, ROWS], bf16, kind="ExternalInput").ap()
    dT_d = nc.dram_tensor("dT", [DIM, DCOLS], bf16, kind="ExternalInput").ap()
    dmax_d = nc.dram_tensor("dmax", [128, UNITS], f32, kind="ExternalOutput").ap()

    def unit_mm(dst, u, half):
        m, d = u // CSHARD, u % CSHARD
        col = d * 1024 + half * 512
        nc.tensor.matmul(
            dst,
            qT_sb[:, m * 128:(m + 1) * 128],
            dT_sb[:, col:col + 512],
            start=True,
            stop=True,
        )

    with tile.TileContext(nc) as tc:
        with (
            tc.tile_pool(name="const", bufs=1) as cpool,
            tc.tile_pool(name="b1", bufs=6) as b1_pool,
            tc.tile_pool(name="pa", bufs=1, space="PSUM") as pa_pool,
            tc.tile_pool(name="pb", bufs=1, space="PSUM") as pb_pool,
        ):
            qT_sb = cpool.tile([DIM, ROWS], bf16)
            dT_sb = cpool.tile([DIM, DCOLS], bf16)
            wsb = cpool.tile([128, 512], bf16)
            # 4 rotating output tiles so consecutive DVE ops have no WAW
            # hazard on a shared tile (a shared tile serializes each op on
            # its predecessor's completion = pipe-drain, ~0.9us/op)
            dmax_sb = [
                cpool.tile([128, UNITS // 4], f32, name=f"dmax{r}") for r in range(4)
            ]
            nc.gpsimd.memset(wsb[:], 0.0)

            A = pa_pool.tile([128, 4, 512], f32)   # A-half slots, banks 0-3
            Bp = pb_pool.tile([128, 3, 512], f32)  # B-half slots, banks 4-6
            Wp = pb_pool.tile([128, 512], f32)     # bank 7: HAM warm-keeper only

            # HAM warm-up on zeros first — only needs wsb, runs during the
            # DMA fill. Wp is PE-private (never read), so warm-keeper matmuls
            # carry no cross-engine waits and never block the PE FIFO.
            for _ in range(8):
                nc.tensor.matmul(Wp[:], wsb[:, 0:128], wsb[:], start=True, stop=True)

            # DMA staging: what the first groups need first, then big chunks.
            nc.sync.dma_start(qT_sb[:, 0:128], qT_d[:, 0:128])
            nc.sync.dma_start(dT_sb[:, 0:1024], dT_d[:, 0:1024])
            nc.sync.dma_start(dT_sb[:, 1024:2048], dT_d[:, 1024:2048])
            nc.sync.dma_start(dT_sb[:, 2048:3072], dT_d[:, 2048:3072])
            nc.gpsimd.dma_start(qT_sb[:, 128:2048], qT_d[:, 128:2048])
            nc.sync.dma_start(dT_sb[:, 3072:4096], dT_d[:, 3072:4096])
            nc.sync.dma_start(dT_sb[:, 4096:6144], dT_d[:, 4096:6144])
            nc.sync.dma_start(dT_sb[:, 6144:8192], dT_d[:, 6144:8192])

            # software pipeline: B-half matmuls + copy run one group ahead;
            # B slots rotate over 3 banks (wrapping pairs use 2 copies)
            def emit_b(g):
                s0, s1 = (2 * g) % 3, (2 * g + 1) % 3
                unit_mm(Bp[:, s0, :], g * GROUP + 0, 1)
                unit_mm(Bp[:, s1, :], g * GROUP + 1, 1)
                t = b1_pool.tile([128, GROUP * 512], f32, tag="b1")
                if s1 == s0 + 1:
                    nc.scalar.copy(t[:], Bp[:, s0:s0 + 2, :])
                else:
                    nc.scalar.copy(t[:, 0:512], Bp[:, s0, :])
                    nc.scalar.copy(t[:, 512:1024], Bp[:, s1, :])
                return t

            def warm_mm():
                # PE-private target: no cross-engine waits, never blocks FIFO
                nc.tensor.matmul(Wp[:], wsb[:, 0:128], wsb[:], start=True, stop=True)

            in1_next = emit_b(0)
            for g in range(NGROUPS):
                s = (g % 2) * 2
                in1 = in1_next
                for j in range(GROUP):
                    unit_mm(A[:, s + j, :], g * GROUP + j, 0)
                warm_mm()
                if g + 1 < NGROUPS:
                    in1_next = emit_b(g + 1)
                    warm_mm()
                r, c = g % 4, (g // 4) * GROUP
                nc.vector._custom_dve(
                    op,
                    out=dmax_sb[r][:, c:c + GROUP],
                    in0=A[:, s:s + GROUP, :],
                    in1=in1[:],
                )

            for r in range(4):
                nc.sync.dma_start(
                    dmax_d[:, r * (UNITS // 4):(r + 1) * (UNITS // 4)], dmax_sb[r][:]
                )

    nc.compile()
    return nc


def _host_inputs(q, d):
    import ml_dtypes

    bf = ml_dtypes.bfloat16
    qT = np.ascontiguousarray(q.transpose(2, 0, 1).reshape(DIM, ROWS)).astype(bf)
    in_maps = []
    for k in range(NCORES):
        dTk = np.ascontiguousarray(
            d[k * CSHARD:(k + 1) * CSHARD].transpose(2, 0, 1).reshape(DIM, DCOLS)
        ).astype(bf)
        in_maps.append({"qT": qT, "dT": dTk})
    return in_maps


def _finish_host(dmaxes, q, offset):
    # dmax[k]: [128 rows, 128 cols], op g (units 2g, 2g+1) wrote cols
    # (g%4)*32 + (g//4)*2 + {0,1}; de-interleave to unit order u = m*8 + d
    g = np.arange(NGROUPS)
    src = (g % 4) * (UNITS // 4) + (g // 4) * GROUP
    perm = np.empty(UNITS, np.int64)
    perm[2 * g] = src
    perm[2 * g + 1] = src + 1
    per_core = []
    for k in range(NCORES):
        m_r_d = dmaxes[k][:, perm].astype(np.float64).reshape(128, MTILES, CSHARD)
        rows_d = m_r_d.transpose(1, 0, 2).reshape(ROWS, CSHARD)
        per_core.append(rows_d.reshape(B, NTOK, CSHARD).sum(axis=1))
    S_mat = np.concatenate(per_core, axis=1)  # [64, 64]
    lengths = (q[:, :, 0] != 0).sum(axis=1).astype(np.float64)
    S_mat = S_mat / lengths[:, None]
    logits = S_mat / TEMPERATURE
    m = logits.max(axis=1, keepdims=True)
    logp = logits - m - np.log(np.exp(logits - m).sum(axis=1, keepdims=True))
    labels = np.arange(B) + offset
    return np.float32(-np.mean(logp[np.arange(B), labels]))


def kernel(**inputs):
    from concourse import bass_utils

    q = np.ascontiguousarray(np.asarray(inputs["query_embeddings"], dtype=np.float32))
    d = np.ascontiguousarray(np.asarray(inputs["doc_embeddings"], dtype=np.float32))
    offset = int(np.asarray(inputs["offset"]))
    assert q.shape == (B, NTOK, DIM) and d.shape == (C, S, DIM)

    if "nc" not in _CACHE:
        _CACHE["nc"] = _build_nc()
    nc = _CACHE["nc"]

    in_maps = _host_inputs(q, d)
    res = bass_utils.run_bass_kernel_spmd(nc, in_maps, core_ids=list(range(NCORES)))
    dmaxes = [res.results[k]["dmax"] for k in range(NCORES)]
    return _finish_host(dmaxes, q, offset)


# revision 12
# speedup vs baseline: 1.1717x; 1.0570x over previous
"""ColBERT MaxSim contrastive loss on 8 Trainium2 NeuronCores.

Sharding: doc-parallel (each core scores ALL 64*32 query tokens against its
8-doc shard = 8192 doc tokens). Per core the work is 128 (m-tile, doc) units
(16 m-tiles of 128 query rows x 8 docs), processed in 64 groups of 2 units:

  - PE: per unit, 2 bf16 matmuls qT[128,128].T @ dT[128,512] -> the unit's
    A-half (tokens 0-511) into the big A PSUM tensor [128, 4, 512] and its
    B-half (tokens 512-1023) into the big B PSUM tensor [128, 4, 512].
    Groups alternate slot pairs {0,1}/{2,3} of each tensor, so the PE fills
    one pair while ScalarE/VectorE drain the other (bank-disjoint).
  - ScalarE: ONE 1024-elem copy per group: B slots -> SBUF.
  - VectorE: ONE custom paged op per group: TT_MAXMAX_PG_ANT streams
    (A_psum[i], B_sbuf[i]) pairs at 1 pair/cycle, keeps a running
    max(pair...) scan that RESETS at each 512-element page (SUB_DIM_DONE
    step state) and writes only page-last values (out_last_subdim_enable)
    -> the 2 units' per-(row, doc) maxes, no accumulator-readout companion.

A dummy matmul per group re-targets the next A slot (overwritten by the real
start=True matmul) purely to keep the PE HAM clock-gate at 8/8.

Host gathers the 8 per-core dmax[128 rows, 128 units] tiles, does the tiny
n-sum over 32 query tokens, length-normalize, and the cross-entropy.
"""

import numpy as np

B, NTOK, DIM = 64, 32, 128
C, S = 64, 1024
NCORES = 8
CSHARD = C // NCORES              # 8 docs per core
ROWS = B * NTOK                   # 2048 score rows
MTILES = ROWS // 128              # 16
DCOLS = CSHARD * S                # 8192 doc-token columns per core
UNITS = MTILES * CSHARD           # 128 (m-tile, doc) units
GROUP = 2                         # units per DVE op / slot pair
NGROUPS = UNITS // GROUP          # 64
TEMPERATURE = 0.02

_CACHE = {}


def _register_ttmax_paged():
    """Custom DVE op: out[p, s] = max_n max(in0[p, s, n], in1[p, s*N + n]).

    Built from Spec(body=Scan(MAX, maxx(Src0, Src1), init=MaxNeg)); the
    lowered [seed, steady] FSM is hand-extended with a SUB_DIM_DONE step
    state that resets the scan flop to the boundary element's pair-max, and
    out_last_subdim_enable so only page-last scan values are written (one
    output per page). Validated bit-exact on hardware for PSUM and SBUF in0.
    """
    import copy

    from concourse import dve_ops as DO
    from concourse.dve_spec import AluOp, MaxNeg, Scan, Spec, Src0, Src1, lower, maxx
    from concourse.dve_uop import AluInp, DveOpSpec, ENABLE, Trigger

    NAME = "TT_MAXMAX_PG_ANT"
    for o in DO.OPS:
        if o.name == NAME:
            return o

    def _ref(in0, in1, c0, c1, c2):
        P = in0.shape[0]
        N = in0.shape[-1]
        Spg = int(np.prod(in0.shape[1:-1]))
        a = np.asarray(in0, np.float32).reshape(P, Spg, N)
        b = np.asarray(in1, np.float32).reshape(P, Spg, N)
        return np.maximum(a, b).max(axis=-1)

    spec = Spec(body=Scan(AluOp.MAX, maxx(Src0, Src1), init=MaxNeg), reference=_ref)
    uops = lower(spec, ver="v3")
    assert len(uops) == 2
    seed, steady = uops
    steady.trigger = (Trigger.SRC_TENSOR_DONE, Trigger.SUB_DIM_DONE, Trigger.NONE)
    steady.next_uop = (0, 2, 0)
    steady.out_last_subdim_enable = ENABLE
    step = copy.deepcopy(steady)
    step.trigger = (Trigger.SRC_TENSOR_DONE, Trigger.SUB_DIM_DONE, Trigger.COUNT)
    step.next_uop = (0, 2, 1)
    step.repeat_count = 1
    dp = step.datapath_config[1]
    dp.op = AluOp.BYPASS
    dp.alu_src0 = AluInp.PREV_ALU_OUT
    dp.alu_src1 = AluInp.PREV_ALU_OUT

    op = DO.DveOp(NAME, spec, subdim=True, uops_sha={})
    DO.OPS.append(op)
    DO.CUSTOM_DVE_SPECS[op.name] = op.spec
    DO._SUB_OPCODE_FOR_NAME[op.name] = DO._CUSTOM_DVE_ROW_BASE + len(DO.OPS) - 1
    ds = DveOpSpec(
        name=NAME,
        opcode=DO.get_dve_sub_opcode(NAME),
        uops=[seed, steady, step],
        rd1_en=True,
    )
    ds.validate("v3")
    op.uops_sha["v3"] = ds.sha("v3")
    DO._COMPILE_CACHE[(NAME, "v3")] = ds
    return op


def _build_nc():
    import concourse.bacc as bacc
    import concourse.tile as tile
    from concourse import mybir

    f32 = mybir.dt.float32
    bf16 = mybir.dt.bfloat16
    op = _register_ttmax_paged()

    nc = bacc.Bacc("TRN2", target_bir_lowering=False, debug=False)
    qT_d = nc.dram_tensor("qT", [DIM, ROWS], bf16, kind="ExternalInput").ap()
    dT_d = nc.dram_tensor("dT", [DIM, DCOLS], bf16, kind="ExternalInput").ap()
    dmax_d = nc.dram_tensor("dmax", [128, UNITS], f32, kind="ExternalOutput").ap()

    def unit_mm(dst, u, half):
        m, d = u // CSHARD, u % CSHARD
        col = d * 1024 + half * 512
        nc.tensor.matmul(
            dst,
            qT_sb[:, m * 128:(m + 1) * 128],
            dT_sb[:, col:col + 512],
            start=True,
            stop=True,
        )

    with tile.TileContext(nc) as tc:
        with (
            tc.tile_pool(name="const", bufs=1) as cpool,
            tc.tile_pool(name="b1", bufs=6) as b1_pool,
            tc.tile_pool(name="pa", bufs=1, space="PSUM") as pa_pool,
            tc.tile_pool(name="pb", bufs=1, space="PSUM") as pb_pool,
        ):
            qT_sb = cpool.tile([DIM, ROWS], bf16)
            dT_sb = cpool.tile([DIM, DCOLS], bf16)
            wsb = cpool.tile([128, 512], bf16)
            # 4 rotating output tiles so consecutive DVE ops have no WAW
            # hazard on a shared tile (a shared tile serializes each op on
            # its predecessor's completion = pipe-drain, ~0.9us/op)
            dmax_sb = [
                cpool.tile([128, UNITS // 4], f32, name=f"dmax{r}") for r in range(4)
            ]
            nc.gpsimd.memset(wsb[:], 0.0)

            # Two separate A tensors (2 banks each): the custom DVE op's APs
            # are dependency-tracked at TENSOR granularity, so each op must
            # touch only its own pair, not a slice of a shared tensor.
            A2 = [
                pa_pool.tile([128, 2, 512], f32, name=f"A{i}") for i in range(2)
            ]
            Bp = pb_pool.tile([128, 3, 512], f32)  # B-half slots, banks 4-6
            Wp = pb_pool.tile([128, 512], f32)     # bank 7: HAM warm-keeper only

            # HAM warm-up on zeros first — only needs wsb, runs during the
            # DMA fill. Wp is PE-private (never read), so warm-keeper matmuls
            # carry no cross-engine waits and never block the PE FIFO.
            for _ in range(8):
                nc.tensor.matmul(Wp[:], wsb[:, 0:128], wsb[:], start=True, stop=True)

            # DMA staging: what the first groups need first, then big chunks.
            nc.sync.dma_start(qT_sb[:, 0:128], qT_d[:, 0:128])
            nc.sync.dma_start(dT_sb[:, 0:1024], dT_d[:, 0:1024])
            nc.sync.dma_start(dT_sb[:, 1024:2048], dT_d[:, 1024:2048])
            nc.sync.dma_start(dT_sb[:, 2048:3072], dT_d[:, 2048:3072])
            nc.gpsimd.dma_start(qT_sb[:, 128:2048], qT_d[:, 128:2048])
            nc.sync.dma_start(dT_sb[:, 3072:4096], dT_d[:, 3072:4096])
            nc.sync.dma_start(dT_sb[:, 4096:6144], dT_d[:, 4096:6144])
            nc.sync.dma_start(dT_sb[:, 6144:8192], dT_d[:, 6144:8192])

            # software pipeline: B-half matmuls + copy run one group ahead;
            # B slots rotate over 3 banks (wrapping pairs use 2 copies)
            def emit_b(g):
                s0, s1 = (2 * g) % 3, (2 * g + 1) % 3
                unit_mm(Bp[:, s0, :], g * GROUP + 0, 1)
                unit_mm(Bp[:, s1, :], g * GROUP + 1, 1)
                t = b1_pool.tile([128, GROUP * 512], f32, tag="b1")
                if s1 == s0 + 1:
                    nc.scalar.copy(t[:], Bp[:, s0:s0 + 2, :])
                else:
                    nc.scalar.copy(t[:, 0:512], Bp[:, s0, :])
                    nc.scalar.copy(t[:, 512:1024], Bp[:, s1, :])
                return t

            def warm_mm():
                # PE-private target: no cross-engine waits, never blocks FIFO
                nc.tensor.matmul(Wp[:], wsb[:, 0:128], wsb[:], start=True, stop=True)

            in1_next = emit_b(0)
            for g in range(NGROUPS):
                A = A2[g % 2]
                in1 = in1_next
                for j in range(GROUP):
                    unit_mm(A[:, j, :], g * GROUP + j, 0)
                warm_mm()
                if g + 1 < NGROUPS:
                    in1_next = emit_b(g + 1)
                    warm_mm()
                r, c = g % 4, (g // 4) * GROUP
                nc.vector._custom_dve(
                    op,
                    out=dmax_sb[r][:, c:c + GROUP],
                    in0=A[:, :, :],
                    in1=in1[:],
                )

            for r in range(4):
                nc.sync.dma_start(
                    dmax_d[:, r * (UNITS // 4):(r + 1) * (UNITS // 4)], dmax_sb[r][:]
                )

    nc.compile()
    return nc


def _host_inputs(q, d):
    import ml_dtypes

    bf = ml_dtypes.bfloat16
    qT = np.ascontiguousarray(q.transpose(2, 0, 1).reshape(DIM, ROWS)).astype(bf)
    in_maps = []
    for k in range(NCORES):
        dTk = np.ascontiguousarray(
            d[k * CSHARD:(k + 1) * CSHARD].transpose(2, 0, 1).reshape(DIM, DCOLS)
        ).astype(bf)
        in_maps.append({"qT": qT, "dT": dTk})
    return in_maps


def _finish_host(dmaxes, q, offset):
    # dmax[k]: [128 rows, 128 cols], op g (units 2g, 2g+1) wrote cols
    # (g%4)*32 + (g//4)*2 + {0,1}; de-interleave to unit order u = m*8 + d
    g = np.arange(NGROUPS)
    src = (g % 4) * (UNITS // 4) + (g // 4) * GROUP
    perm = np.empty(UNITS, np.int64)
    perm[2 * g] = src
    perm[2 * g + 1] = src + 1
    per_core = []
    for k in range(NCORES):
        m_r_d = dmaxes[k][:, perm].astype(np.float64).reshape(128, MTILES, CSHARD)
        rows_d = m_r_d.transpose(1, 0, 2).reshape(ROWS, CSHARD)
        per_core.append(rows_d.reshape(B, NTOK, CSHARD).sum(axis=1))
    S_mat = np.concatenate(per_core, axis=1)  # [64, 64]
    lengths = (q[:, :, 0] != 0).sum(axis=1).astype(np.float64)
    S_mat = S_mat / lengths[:, None]
    logits = S_mat / TEMPERATURE
    m = logits.max(axis=1, keepdims=True)
    logp = logits - m - np.log(np.exp(logits - m).sum(axis=1, keepdims=True))
    labels = np.arange(B) + offset
    return np.float32(-np.mean(logp[np.arange(B), labels]))


def kernel(**inputs):
    from concourse import bass_utils

    q = np.ascontiguousarray(np.asarray(inputs["query_embeddings"], dtype=np.float32))
    d = np.ascontiguousarray(np.asarray(inputs["doc_embeddings"], dtype=np.float32))
    offset = int(np.asarray(inputs["offset"]))
    assert q.shape == (B, NTOK, DIM) and d.shape == (C, S, DIM)

    if "nc" not in _CACHE:
        _CACHE["nc"] = _build_nc()
    nc = _CACHE["nc"]

    in_maps = _host_inputs(q, d)
    res = bass_utils.run_bass_kernel_spmd(nc, in_maps, core_ids=list(range(NCORES)))
    dmaxes = [res.results[k]["dmax"] for k in range(NCORES)]
    return _finish_host(dmaxes, q, offset)


# revision 16
# speedup vs baseline: 1.6034x; 1.3685x over previous
"""ColBERT MaxSim contrastive loss on 8 Trainium2 NeuronCores.

Sharding: doc-parallel (each core scores ALL 64*32 query tokens against its
8-doc shard = 8192 doc tokens). Per core the work is 128 (m-tile, doc) units
(16 m-tiles of 128 query rows x 8 docs), processed in 64 groups of 2 units:

  - PE: per unit, 2 bf16 matmuls qT[128,128].T @ dT[128,512] -> the unit's
    A-half (tokens 0-511) into the big A PSUM tensor [128, 4, 512] and its
    B-half (tokens 512-1023) into the big B PSUM tensor [128, 4, 512].
    Groups alternate slot pairs {0,1}/{2,3} of each tensor, so the PE fills
    one pair while ScalarE/VectorE drain the other (bank-disjoint).
  - ScalarE: ONE 1024-elem copy per group: B slots -> SBUF.
  - VectorE: ONE custom paged op per group: TT_MAXMAX_PG_ANT streams
    (A_psum[i], B_sbuf[i]) pairs at 1 pair/cycle, keeps a running
    max(pair...) scan that RESETS at each 512-element page (SUB_DIM_DONE
    step state) and writes only page-last values (out_last_subdim_enable)
    -> the 2 units' per-(row, doc) maxes, no accumulator-readout companion.

A dummy matmul per group re-targets the next A slot (overwritten by the real
start=True matmul) purely to keep the PE HAM clock-gate at 8/8.

Host gathers the 8 per-core dmax[128 rows, 128 units] tiles, does the tiny
n-sum over 32 query tokens, length-normalize, and the cross-entropy.
"""

import numpy as np

B, NTOK, DIM = 64, 32, 128
C, S = 64, 1024
NCORES = 8
CSHARD = C // NCORES              # 8 docs per core
ROWS = B * NTOK                   # 2048 score rows
MTILES = ROWS // 128              # 16
DCOLS = CSHARD * S                # 8192 doc-token columns per core
UNITS = MTILES * CSHARD           # 128 (m-tile, doc) units
GROUP = 2                         # units per DVE op / slot pair
NGROUPS = UNITS // GROUP          # 64
TEMPERATURE = 0.02

_CACHE = {}


def _register_ttmax_paged():
    """Custom DVE op: out[p, s] = max_n max(in0[p, s, n], in1[p, s*N + n]).

    Built from Spec(body=Scan(MAX, maxx(Src0, Src1), init=MaxNeg)); the
    lowered [seed, steady] FSM is hand-extended with a SUB_DIM_DONE step
    state that resets the scan flop to the boundary element's pair-max, and
    out_last_subdim_enable so only page-last scan values are written (one
    output per page). Validated bit-exact on hardware for PSUM and SBUF in0.
    """
    import copy

    from concourse import dve_ops as DO
    from concourse.dve_spec import AluOp, MaxNeg, Scan, Spec, Src0, Src1, lower, maxx
    from concourse.dve_uop import AluInp, DveOpSpec, ENABLE, Trigger

    NAME = "TT_MAXMAX_PG_ANT"
    for o in DO.OPS:
        if o.name == NAME:
            return o

    def _ref(in0, in1, c0, c1, c2):
        P = in0.shape[0]
        N = in0.shape[-1]
        Spg = int(np.prod(in0.shape[1:-1]))
        a = np.asarray(in0, np.float32).reshape(P, Spg, N)
        b = np.asarray(in1, np.float32).reshape(P, Spg, N)
        return np.maximum(a, b).max(axis=-1)

    spec = Spec(body=Scan(AluOp.MAX, maxx(Src0, Src1), init=MaxNeg), reference=_ref)
    uops = lower(spec, ver="v3")
    assert len(uops) == 2
    seed, steady = uops
    steady.trigger = (Trigger.SRC_TENSOR_DONE, Trigger.SUB_DIM_DONE, Trigger.NONE)
    steady.next_uop = (0, 2, 0)
    steady.out_last_subdim_enable = ENABLE
    step = copy.deepcopy(steady)
    step.trigger = (Trigger.SRC_TENSOR_DONE, Trigger.SUB_DIM_DONE, Trigger.COUNT)
    step.next_uop = (0, 2, 1)
    step.repeat_count = 1
    dp = step.datapath_config[1]
    dp.op = AluOp.BYPASS
    dp.alu_src0 = AluInp.PREV_ALU_OUT
    dp.alu_src1 = AluInp.PREV_ALU_OUT

    op = DO.DveOp(NAME, spec, subdim=True, uops_sha={})
    DO.OPS.append(op)
    DO.CUSTOM_DVE_SPECS[op.name] = op.spec
    DO._SUB_OPCODE_FOR_NAME[op.name] = DO._CUSTOM_DVE_ROW_BASE + len(DO.OPS) - 1
    ds = DveOpSpec(
        name=NAME,
        opcode=DO.get_dve_sub_opcode(NAME),
        uops=[seed, steady, step],
        rd1_en=True,
    )
    ds.validate("v3")
    op.uops_sha["v3"] = ds.sha("v3")
    DO._COMPILE_CACHE[(NAME, "v3")] = ds
    return op


def _build_nc():
    import concourse.bacc as bacc
    import concourse.tile as tile
    from concourse import mybir

    f32 = mybir.dt.float32
    bf16 = mybir.dt.bfloat16
    op = _register_ttmax_paged()

    nc = bacc.Bacc("TRN2", target_bir_lowering=False, debug=False)
    qT_d = nc.dram_tensor("qT", [DIM, ROWS], bf16, kind="ExternalInput").ap()
    dT_d = nc.dram_tensor("dT", [DIM, DCOLS], bf16, kind="ExternalInput").ap()
    dmax_d = nc.dram_tensor("dmax", [128, UNITS], f32, kind="ExternalOutput").ap()

    def unit_mm(dst, u, half):
        m, d = u // CSHARD, u % CSHARD
        col = d * 1024 + half * 512
        nc.tensor.matmul(
            dst,
            qT_sb[:, m * 128:(m + 1) * 128],
            dT_sb[:, col:col + 512],
            start=True,
            stop=True,
        )

    with tile.TileContext(nc) as tc:
        with (
            tc.tile_pool(name="const", bufs=1) as cpool,
            tc.tile_pool(name="b1", bufs=6) as b1_pool,
            tc.tile_pool(name="pa", bufs=1, space="PSUM") as pa_pool,
            tc.tile_pool(name="pb", bufs=1, space="PSUM") as pb_pool,
        ):
            qT_sb = cpool.tile([DIM, ROWS], bf16)
            dT_sb = cpool.tile([DIM, DCOLS], bf16)
            wsb = cpool.tile([128, 512], bf16)
            # 4 rotating output tiles so consecutive DVE ops have no WAW
            # hazard on a shared tile (a shared tile serializes each op on
            # its predecessor's completion = pipe-drain, ~0.9us/op)
            dmax_sb = [
                cpool.tile([128, UNITS // 2], f32, name=f"dmax{r}") for r in range(2)
            ]
            nc.gpsimd.memset(wsb[:], 0.0)

            # Separate A/B tensors per pipeline phase (2 banks each): the
            # custom DVE op's APs are dependency-tracked at TENSOR
            # granularity, so each op must touch only its own pair tensor,
            # and double-buffering needs physically distinct tensors.
            A2 = [
                pa_pool.tile([128, 2, 512], f32, name=f"A{i}") for i in range(2)
            ]
            B2 = [
                pb_pool.tile([128, 2, 512], f32, name=f"B{i}") for i in range(2)
            ]

            # HAM warm-up on zeros first — only needs wsb, runs during the
            # DMA fill; group 0's real start=True matmul overwrites the slot.
            for _ in range(8):
                nc.tensor.matmul(
                    A2[0][:, 0, :], wsb[:, 0:128], wsb[:], start=True, stop=True
                )

            # DMA staging: what the first groups need first, then big chunks.
            nc.sync.dma_start(qT_sb[:, 0:128], qT_d[:, 0:128])
            nc.sync.dma_start(dT_sb[:, 0:1024], dT_d[:, 0:1024])
            nc.sync.dma_start(dT_sb[:, 1024:2048], dT_d[:, 1024:2048])
            nc.sync.dma_start(dT_sb[:, 2048:3072], dT_d[:, 2048:3072])
            nc.gpsimd.dma_start(qT_sb[:, 128:2048], qT_d[:, 128:2048])
            nc.sync.dma_start(dT_sb[:, 3072:4096], dT_d[:, 3072:4096])
            nc.sync.dma_start(dT_sb[:, 4096:6144], dT_d[:, 4096:6144])
            nc.sync.dma_start(dT_sb[:, 6144:8192], dT_d[:, 6144:8192])

            # software pipeline: B-half matmuls + copy run one group ahead.
            # A dummy matmul before each real pair targets the same slot the
            # real start=True matmul overwrites — same dependency, no extra
            # serialization, keeps the PE HAM clock-gate at 8/8.
            def emit_b(g):
                Bt = B2[g % 2]
                nc.tensor.matmul(
                    Bt[:, 0, :], wsb[:, 0:128], wsb[:], start=True, stop=True
                )
                unit_mm(Bt[:, 0, :], g * GROUP + 0, 1)
                unit_mm(Bt[:, 1, :], g * GROUP + 1, 1)
                t = b1_pool.tile([128, GROUP * 512], f32, tag="b1")
                nc.scalar.copy(t[:], Bt[:, :, :])
                return t

            in1_next = emit_b(0)
            for g in range(NGROUPS):
                A = A2[g % 2]
                in1 = in1_next
                nc.tensor.matmul(
                    A[:, 0, :], wsb[:, 0:128], wsb[:], start=True, stop=True
                )
                for j in range(GROUP):
                    unit_mm(A[:, j, :], g * GROUP + j, 0)
                if g + 1 < NGROUPS:
                    in1_next = emit_b(g + 1)
                r, c = g % 2, (g // 2) * GROUP
                nc.vector._custom_dve(
                    op,
                    out=dmax_sb[r][:, c:c + GROUP],
                    in0=A[:, :, :],
                    in1=in1[:],
                )

            nc.sync.dma_start(dmax_d[:, 0:UNITS // 2], dmax_sb[0][:])
            nc.gpsimd.dma_start(dmax_d[:, UNITS // 2:UNITS], dmax_sb[1][:])

    nc.compile()
    return nc


def _host_inputs(q, d):
    import ml_dtypes

    bf = ml_dtypes.bfloat16
    qT = np.ascontiguousarray(q.transpose(2, 0, 1).reshape(DIM, ROWS)).astype(bf)
    in_maps = []
    for k in range(NCORES):
        dTk = np.ascontiguousarray(
            d[k * CSHARD:(k + 1) * CSHARD].transpose(2, 0, 1).reshape(DIM, DCOLS)
        ).astype(bf)
        in_maps.append({"qT": qT, "dT": dTk})
    return in_maps


def _finish_host(dmaxes, q, offset):
    # dmax[k]: [128 rows, 128 cols], op g (units 2g, 2g+1) wrote cols
    # (g%2)*64 + (g//2)*2 + {0,1}; de-interleave to unit order u = m*8 + d
    g = np.arange(NGROUPS)
    src = (g % 2) * (UNITS // 2) + (g // 2) * GROUP
    perm = np.empty(UNITS, np.int64)
    perm[2 * g] = src
    perm[2 * g + 1] = src + 1
    per_core = []
    for k in range(NCORES):
        m_r_d = dmaxes[k][:, perm].astype(np.float64).reshape(128, MTILES, CSHARD)
        rows_d = m_r_d.transpose(1, 0, 2).reshape(ROWS, CSHARD)
        per_core.append(rows_d.reshape(B, NTOK, CSHARD).sum(axis=1))
    S_mat = np.concatenate(per_core, axis=1)  # [64, 64]
    lengths = (q[:, :, 0] != 0).sum(axis=1).astype(np.float64)
    S_mat = S_mat / lengths[:, None]
    logits = S_mat / TEMPERATURE
    m = logits.max(axis=1, keepdims=True)
    logp = logits - m - np.log(np.exp(logits - m).sum(axis=1, keepdims=True))
    labels = np.arange(B) + offset
    return np.float32(-np.mean(logp[np.arange(B), labels]))


def kernel(**inputs):
    from concourse import bass_utils

    q = np.ascontiguousarray(np.asarray(inputs["query_embeddings"], dtype=np.float32))
    d = np.ascontiguousarray(np.asarray(inputs["doc_embeddings"], dtype=np.float32))
    offset = int(np.asarray(inputs["offset"]))
    assert q.shape == (B, NTOK, DIM) and d.shape == (C, S, DIM)

    if "nc" not in _CACHE:
        _CACHE["nc"] = _build_nc()
    nc = _CACHE["nc"]

    in_maps = _host_inputs(q, d)
    res = bass_utils.run_bass_kernel_spmd(nc, in_maps, core_ids=list(range(NCORES)))
    dmaxes = [res.results[k]["dmax"] for k in range(NCORES)]
    return _finish_host(dmaxes, q, offset)
